# revision 1
# baseline (speedup 1.0000x reference)
"""Trainium2 Bass kernel for BaseTextureNCA (neural cellular automaton step).

Math:
  y  = depthwise 3x3 conv of x with 4 fixed filters (circular pad)   [b,48,H,W]
  h  = relu(W1 @ y + b1)                                             [b,96,H,W]
  dy = W2 @ h                                                        [b,12,H,W]
  out = x + dy * floor(rand_u + 0.5)

Kernel formulation (per core = one batch image):
  - Fold the fixed filters into W1: h = relu(conv3x3(x, W1c) + b1) with
    W1c[o,c,ky,kx] = sum_f W1[o, 4c+f] * F[f,ky,kx].
  - Prologue stages x into xpad2 [C, H+2, W+2] in DRAM with both circular
    pads materialized (built through SBUF with VectorE doing the padding).
  - conv3x3 as 2 accumulating PE matmuls per output row over an SBUF chunk
    buffer xb holding 6 vertically-shifted window copies of xpad2
    (3 dy-shifts x 2 one-element-offset blocks); horizontal shifts are
    free-dim offsets into the padded rows.
  - The stochastic mask is folded into conv1 as one extra contraction row t
    with t = -1e6 where rand_u < 0.5 else 0: relu(pre + t) == relu(pre)*mask.
  - conv2 appends a 12x12 identity block (K=108) so PSUM holds the final
    x + dy*mask directly; two rows share a 2-bank PSUM tile and one
    VectorE copy per pair evacuates PSUM -> SBUF -> HBM.
  - Matmuls run as float32r (fp32 storage, relaxed multiply, full PE rate).
  - Structure keeps per-instruction sync-wait fan-in within ISA budgets
    (1 for DMA, 2 for matmul): single DMA-completion semaphore lane, DMAs
    only ever target buffers whose last non-DMA toucher is one engine.
"""

import os
import sys

import numpy as np

for _p in ("/opt/trn_rl_repo", os.path.expanduser("~/.axon_site/_ro/trn_rl_repo")):
    if os.path.isdir(os.path.join(_p, "concourse")) and _p not in sys.path:
        sys.path.insert(0, _p)

import concourse.bass as bass
import concourse.mybir as mybir
import concourse.tile as tile
import concourse.tile_sem_assignment as _tsa
from contextlib import ExitStack

# Funnel all DMA completion semaphores onto one HWDGE + one SWDGE lane.
# Same-ring DMAs complete (sem-inc) in FIFO issue order, so a single
# counting lane is sound, and it caps the per-instruction sync-wait
# fan-in (TRN2 ISA allows only 1 wait on a DMA, 2 on a matmul; every
# distinct lane costs a wait slot).
_tsa.NUM_HWDGE_SEMS = 1
_tsa.NUM_SWDGE_GLOBAL_SEMS = 1

C = 12
HID = 96
NCORES = 8
K1 = 73          # 6 shifted x copies (72 partitions) + 1 mask row
K2 = 36
KC2 = HID + C    # conv2 contraction: [W2^T; I12] -> 108
MC2 = 32         # conv2 weight block width in the wall (12 used)
BIG_NEG = -1.0e6
FP = mybir.dt.float32

_IDENT = np.array([[0., 0., 0.], [0., 1., 0.], [0., 0., 0.]], np.float32)
_SOBX = np.array([[-1., 0., 1.], [-2., 0., 2.], [-1., 0., 1.]], np.float32)
_SOBY = _SOBX.T
_LAP = np.array([[1., 2., 1.], [2., -12., 2.], [1., 2., 1.]], np.float32)
FILTERS = np.stack([_IDENT, _SOBX, _SOBY, _LAP])  # [4,3,3]

WALLF = 2 * HID + MC2  # packed weight-wall free size (224)


def host_weights(w1_w, w1_b, w2_w):
    """Pack all lhsT weight mats into one [128, 224] wall + the bias.

    wall[0:73, 0:96]    = wp1 (conv1 pass 1: taps dx=-1 blk0, dx=0 blk1,
                          + mask row)
    wall[0:36, 96:192]  = wp2 (conv1 pass 2: taps dx=+1 on blk0)
    wall[0:96, 192:224] = W2^T zero-padded to 32 cols
    """
    w1r = np.asarray(w1_w, np.float32).reshape(HID, C, 4)
    w1c = np.einsum("ocf,fab->ocab", w1r, FILTERS)  # [96,12,3,3]

    wall = np.zeros((128, WALLF), np.float32)
    for v in range(3):
        for c in range(C):
            wall[v * C + c, 0:HID] = w1c[:, c, v, 0]        # (dy=v-1, dx=-1)
            wall[36 + v * C + c, 0:HID] = w1c[:, c, v, 1]   # (dy=v-1, dx= 0)
            wall[v * C + c, HID:2 * HID] = w1c[:, c, v, 2]  # (dy=v-1, dx=+1)
    wall[72, 0:HID] = 1.0                                   # mask-penalty row

    wall[:HID, 2 * HID:2 * HID + C] = np.asarray(w2_w, np.float32).T
    wall[HID:KC2, 2 * HID:2 * HID + C] = np.eye(C, dtype=np.float32)
    b1 = np.asarray(w1_b, np.float32).reshape(HID, 1).copy()
    return wall, b1


def build_nc(H=512, W=512, R=16, f32r=True, act_pairs=8):
    """Build the per-core Bass program.

    R: rows per processing chunk. act_pairs: of the R//2 row-pairs per
    chunk, how many use ScalarE for relu+bias (rest on VectorE).
    """
    PW = W + 2
    RPP = max(1, H // 128)     # rand_u rows per partition in the t image
    PT = H // RPP
    PB = 64                    # prologue rows per pass
    assert H % R == 0 and R % 4 == 0 and R % RPP == 0 and H % PB == 0
    MMDT = mybir.dt.float32r if f32r else FP

    nc = bass.Bass()
    x_d = nc.declare_dram_parameter("x", [C, H, W], FP, isOutput=False)
    u_d = nc.declare_dram_parameter("u", [H, W], FP, isOutput=False)
    wall_d = nc.declare_dram_parameter("wall", [128, WALLF], MMDT,
                                       isOutput=False)
    b1_d = nc.declare_dram_parameter("b1", [HID, 1], FP, isOutput=False)
    out_d = nc.declare_dram_parameter("out", [C, H, W], FP, isOutput=True)

    AF = mybir.ActivationFunctionType
    AL = mybir.AluOpType

    with tile.TileContext(nc) as tc:
        with ExitStack() as ctx:
            dpool = ctx.enter_context(
                tc.tile_pool(name="dram", bufs=1, space="DRAM"))
            xpad = dpool.tile([C, (H + 2) * PW], MMDT, tag="xpad")
            xp2 = xpad[:, :].rearrange("c (r w) -> c r w", w=PW)
            xp_t = xpad[:, :].tensor
            xp_base = xpad[:, :].offset

            consts = ctx.enter_context(tc.tile_pool(name="consts", bufs=1))
            tpool = ctx.enter_context(tc.tile_pool(name="timg", bufs=1))

            # ---- Prologue B first: weights + mask image, so chunk 0's
            # dependencies (wall, b1, t_dram) clear while the bulkier
            # xpad2 staging below is still streaming.
            wall_sb = consts.tile([128, WALLF], MMDT, tag="wall")
            nc.sync.dma_start(wall_sb[:], wall_d[:, :])
            wp1_sb = wall_sb[0:K1, 0:HID]
            wp2_sb = wall_sb[0:K2, HID:2 * HID]
            wc2_sb = wall_sb[0:KC2, 2 * HID:2 * HID + C]
            b1_sb = consts.tile([HID, 1], FP, tag="b1")
            nc.sync.dma_start(b1_sb[:], b1_d[:, :])

            u_sb = tpool.tile([PT, RPP * W], FP, tag="u")
            nc.sync.dma_start(
                u_sb[:], u_d[:, :].rearrange("(p q) w -> p (q w)", q=RPP))
            t_sb = tpool.tile([PT, RPP * W], MMDT, tag="t")
            nc.vector.tensor_scalar(
                t_sb[:], u_sb[:], 0.5, BIG_NEG, op0=AL.is_lt, op1=AL.mult)
            t_dram = dpool.tile([PT, RPP * W], MMDT, tag="t_dram")
            nc.gpsimd.dma_start(t_dram[:, :], t_sb[:])
            tdv = t_dram[:, :].rearrange("p (q w) -> (p q) w", w=W)

            # ---- Prologue A: build xpad2 = circularly padded x in DRAM.
            with tc.tile_pool(name="prolog", bufs=3) as ppool:
                for p0 in range(0, H, PB):
                    s1 = ppool.tile([PB, C * W], FP, tag="s1")
                    nc.sync.dma_start(
                        s1[:, :].rearrange("p (c w) -> p c w", w=W),
                        x_d[:, p0:p0 + PB, :].transpose([1, 0, 2]))
                    s2 = ppool.tile([PB, C * PW], MMDT, tag="s2")
                    s1v = s1[:, :].rearrange("p (c w) -> p c w", w=W)
                    s2v = s2[:, :].rearrange("p (c w) -> p c w", w=PW)
                    nc.vector.tensor_copy(s2v[:, :, 1:W + 1], s1v[:, :, :])
                    nc.vector.tensor_copy(s2v[:, :, 0:1],
                                          s1v[:, :, W - 1:W])
                    nc.vector.tensor_copy(s2v[:, :, W + 1:W + 2],
                                          s1v[:, :, 0:1])
                    # Store via SWDGE: its wait on the DVE padding must
                    # not stall the SP queue issuing the next pass load.
                    nc.gpsimd.dma_start(
                        xp2[:, p0 + 1:p0 + PB + 1, :].transpose([1, 0, 2]),
                        s2[:, :].rearrange("p (c w) -> p c w", w=PW))
            # Vertical wrap rows: row 0 <- x row H-1, row H+1 <- x row 0.
            nc.gpsimd.dma_start(xp2[:, 0:1, :], xp2[:, H:H + 1, :])
            nc.gpsimd.dma_start(xp2[:, H + 1:H + 2, :], xp2[:, 1:2, :])

            xpool = ctx.enter_context(tc.tile_pool(name="xbuf", bufs=2))
            hpool = ctx.enter_context(tc.tile_pool(name="h", bufs=2))
            opool = ctx.enter_context(tc.tile_pool(name="ostage", bufs=2))
            ph_pool = ctx.enter_context(
                tc.tile_pool(name="psum_h", bufs=2, space="PSUM"))
            po_pool = ctx.enter_context(
                tc.tile_pool(name="psum_o", bufs=2, space="PSUM"))

            n_chunks = H // R
            # Interior chunks first: chunks 0 and last read the vertical
            # wrap rows written at the very end of the prologue.
            order = list(range(1, n_chunks - 1)) + [0, n_chunks - 1]
            for ci in order:
                r0 = ci * R
                xb = xpool.tile([K1, R * PW], MMDT, tag="xb")

                def xv(p0, p1):
                    return xb[p0:p1, :].rearrange("p (r c) -> p r c", c=PW)

                # Two window loads from xpad2 (verbatim and +1 element):
                # src dims (g, c, flat R*PW); dst partition = g*12 + c.
                # Position p of dst row r = x[c, r0+r+g-1, p-1-blk].
                for blk in range(2):
                    cnt = R * PW - (1 if (blk and r0 == H - R) else 0)
                    src = bass.AP(
                        xp_t, xp_base + r0 * PW + blk,
                        [[PW, 3], [(H + 2) * PW, C], [1, cnt]])
                    nc.sync.dma_start(
                        out=xb[blk * 36:blk * 36 + 36, 0:cnt], in_=src)

                # Mask rows into partition 72 (aligned with pass-1 offset).
                nc.sync.dma_start(
                    out=xb[K1 - 1:K1, :].rearrange(
                        "p (r c) -> p r c", c=PW)[:, 0:R, 0:W],
                    in_=tdv[r0:r0 + R, :])

                # h chunk; partitions 96:108 hold x rows for the residual
                # (the I12 block of the conv2 weights adds them back).
                # Issued on the SWDGE (Pool) queue: its waits (h WAR/WAW
                # vs relu + conv2 readers) must not stall the SP queue
                # that prefetches the next chunks' loads.
                h = hpool.tile([KC2, R * W], MMDT, tag="h")
                nc.gpsimd.dma_start(
                    out=h[HID:KC2, :],
                    in_=xp2[:, r0 + 1:r0 + 1 + R, 1:W + 1])

                # Interleave conv1 (pair rp) with conv2+evac (pair rp-2)
                # so PE alternates producer/consumer work and ACT/DVE run
                # continuously instead of phase-by-phase.
                HR = R // 2
                NP = R // 2
                osts = [None, None]

                def conv1_pair(rp):
                    ph = ph_pool.tile([HID, 2 * W], FP, tag="ph",
                                      name=f"ph_{ci}_{rp}")
                    for j in range(2):
                        O = (rp * 2 + j) * PW
                        nc.tensor.matmul(
                            ph[:, j * W:(j + 1) * W],
                            wp1_sb, xb[0:K1, O:O + W],
                            start=True, stop=False)
                        nc.tensor.matmul(
                            ph[:, j * W:(j + 1) * W],
                            wp2_sb, xb[0:K2, O + 2:O + 2 + W],
                            start=False, stop=True)
                    hs = h[0:HID, rp * 2 * W:(rp + 1) * 2 * W]
                    use_act = (rp % 2 == 0) if act_pairs == 4 \
                        else rp < act_pairs
                    if use_act:
                        nc.scalar.activation(
                            hs, ph[:], AF.Relu, bias=b1_sb[:, 0:1])
                    else:
                        nc.vector.tensor_scalar(
                            hs, ph[:], b1_sb[:, 0:1], 0.0,
                            op0=AL.add, op1=AL.max)

                def conv2_pair(g):
                    half = (g * 2) // HR
                    if osts[half] is None:
                        osts[half] = opool.tile([C, HR * W], FP, tag="ost",
                                                name=f"ost_{ci}_{half}")
                    ost = osts[half]
                    gl = g - half * (HR // 2)
                    po = po_pool.tile([C, 2 * W], FP, tag="po",
                                      name=f"po_{ci}_{g}")
                    for j in range(2):
                        r = g * 2 + j
                        nc.tensor.matmul(
                            po[:, j * W:(j + 1) * W],
                            wc2_sb, h[0:KC2, r * W:(r + 1) * W],
                            start=True, stop=True)
                    nc.vector.tensor_copy(
                        ost[0:C, gl * 2 * W:(gl + 1) * 2 * W], po[:])
                    if (g * 2 + 2) == (half + 1) * HR:
                        nc.gpsimd.dma_start(
                            out=out_d[:, r0 + half * HR:
                                      r0 + (half + 1) * HR, :],
                            in_=ost[0:C, :])

                for rp in range(NP):
                    conv1_pair(rp)
                    if rp >= 2:
                        conv2_pair(rp - 2)
                conv2_pair(NP - 2)
                conv2_pair(NP - 1)

    return nc


_DMA_TYPES = ("InstDMACopy", "InstDMA", "InstDmaTransposeAnt",
              "InstDMAGatherAnt", "InstDMAScatterAddAnt")


def _wait_budget(inst):
    return 1


def _split_sync_waits(nc):
    """Move excess per-instruction sem waits onto preceding NoOps.

    The TRN2 ISA caps sync-wait commands per instruction (1 for the DMA
    pseudo-instructions, ~2 elsewhere); walrus refuses to compile above
    the cap. A NoOp on the same engine queue executes its wait in program
    order before the real instruction, so spreading is semantically
    identical.
    """
    import bass_rust

    n = 0
    for fn in nc.m.functions:
        for bb in fn.blocks:
            insts = bb.instructions
            out = []
            for inst in insts:
                si = inst.sync_info
                budget = _wait_budget(inst)
                if si is not None and len(si.on_wait) > budget:
                    waits = list(si.on_wait)
                    excess = waits[:len(waits) - budget]
                    keep = waits[len(waits) - budget:]
                    for w in excess:
                        n += 1
                        nop = mybir.InstNoOp(name=f"wsplit_{n}", ins=[],
                                             outs=[])
                        nop.engine = inst.engine
                        nop.sync_info = bass_rust.SyncInfo(
                            on_wait=[w], on_update=[])
                        out.append(nop)
                    inst.sync_info = bass_rust.SyncInfo(
                        on_wait=keep, on_update=list(si.on_update))
                out.append(inst)
            insts.clear()
            insts.extend(out)
    return n


_NC_CACHE = {}


def _get_nc(**kw):
    key = tuple(sorted(kw.items()))
    if key not in _NC_CACHE:
        nc = build_nc(**kw)
        # Wait-splitting breaks CoreSim's accounting, so it is applied
        # only on the hardware path (here), not inside build_nc.
        _split_sync_waits(nc)
        _NC_CACHE[key] = nc
    return _NC_CACHE[key]


def run(x, w1_w, w1_b, w2_w, rand_u, trace=False, **build_kw):
    """Shard over batch, run on 8 cores, gather. Returns (out, results)."""
    from concourse.bass_utils import run_bass_kernel_spmd

    x = np.ascontiguousarray(np.asarray(x, np.float32))
    rand_u = np.ascontiguousarray(np.asarray(rand_u, np.float32))
    b, c, hh, ww = x.shape
    assert b == NCORES and c == C
    wall, b1 = host_weights(w1_w, w1_b, w2_w)

    nc = _get_nc(H=hh, W=ww, **build_kw)
    in_maps = [
        {
            "x": x[i],
            "u": rand_u[i, 0],
            "wall": wall,
            "b1": b1,
        }
        for i in range(NCORES)
    ]
    res = run_bass_kernel_spmd(nc, in_maps, list(range(NCORES)), trace=trace)
    out = np.stack([res.results[i]["out"] for i in range(NCORES)])
    return out.astype(np.float32), res


def kernel(x, w1_w, w1_b, w2_w, rand_u):
    out, _ = run(x, w1_w, w1_b, w2_w, rand_u)
    return out

